# revision 3
# baseline (speedup 1.0000x reference)
"""Causal single-head attention (B=4, T=4096, C=1024, H=64) on 8 TRN2 cores.

v3: balanced pair decomposition. core = 2*b + h handles batch b, t-half h.
Every core runs the SAME shapes (SPMD):
  - x-prep: cast x f32->bf16 HBM->SBUF (no DRAM round trip), SBUF->SBUF
    DMA-transposes into xT [128, 8, 2048].
  - proj: qkT via wqk-stationary (psum [128,512] chunks) -> qk2q/qk2k bf16
    duplicated across both partition halves (for row-tiled QK pairs);
    v via xT-stationary -> v_sb [128, 16, 66] (ones cols for denominator).
  - AG-1 (after h1 proj): own (k_h1 | v_h1 | q_h1); AG-2: (k_h2 | v_h2 | q_h2).
    Every core reads slot0.k/v (= lower core's k/v) and slot1.q.
  - tri QK: row-tiled pairs (contraction 64 at partitions 0:64 and 64:128
    concurrently), psum [128,2,512], mask add on diag chunks, exp N=1024
    -> attT_tri (512-rounded storage as v2).
  - rect (1024 q-rows x 2048 s): QR = glow*AG1.slot1.q + flag*AG2.slot1.q
    (lower core processes upper's rows [0:1024); upper processes its own
    rows [1024:2048)). Row-tiled pairs vs KR2 (slot0 k, duplicated), exp
    N=1024 -> attT_rect [128, 16, 1024].
  - AV: attT-stationary (FWL), rhs v (N=65): tri -> trind [t,66];
    rect with VR (slot0 v) -> rectnd [8 tiles, 66].
  - AG-3: rectnd f32; both read slot0 (= lower's partial = upper rows
    [2048:3072) contribution).
  - final: tiles 0..7: nd = trind + flag*recv; tiles 8..15: nd = trind +
    flag*rectnd; out = num/den.
"""
import sys

sys.path.insert(0, "/opt/trn_rl_repo")

from contextlib import ExitStack

import numpy as np

import concourse.bass as bass
import concourse.mybir as mybir
import concourse.tile as tile
from concourse import bacc
from concourse.bass_utils import run_bass_kernel_spmd

B, T, C, H = 4, 4096, 1024, 64
P = 128
HALF = T // 2              # 2048 rows per core
NB_C = C // P              # 8 contraction tiles
NT = HALF // P             # 16 own t/s tiles
SCALE = float(H) ** -0.5
NEG = -1e9
F32, BF16 = mybir.dt.float32, mybir.dt.bfloat16
N_CORES = 8
PAIRS = [[2 * b, 2 * b + 1] for b in range(B)]

# triangle attT storage: s-tile i holds local t-cols [TRI_BASE[i], 2048)
TRI_BASE = [(i // 4) * 512 for i in range(NT)]
TRI_W = [HALF - b for b in TRI_BASE]
TRI_OFF = np.concatenate([[0], np.cumsum(TRI_W)]).tolist()
TRI_TOTAL = TRI_OFF[-1]  # 20480

# AG-1/2 payload (bf16): k [64,1024] | v [128,8,66] | q [64,1024]
KV_K = 64 * 1024                 # 65536
KV_V = P * (NT // 2) * (H + 2)   # 67584
KV_Q = 64 * 1024                 # 65536
KVQ_N = KV_K + KV_V + KV_Q
ND_N = P * (NT // 2) * (H + 2)   # rectnd f32 payload elems

_CACHE = {}
BODY_REPEAT = 1
PHASES = set(range(1, 9))
SCHEDULE = None


def build():
    nc = bacc.Bacc("TRN2", target_bir_lowering=False, debug=False,
                   num_devices=N_CORES)
    x = nc.dram_tensor("x", [HALF, C], F32, kind="ExternalInput").ap()
    wq = nc.dram_tensor("wq", [C, H], F32, kind="ExternalInput").ap()
    wk = nc.dram_tensor("wk", [C, H], F32, kind="ExternalInput").ap()
    wv = nc.dram_tensor("wv", [C, H], F32, kind="ExternalInput").ap()
    flag = nc.dram_tensor("flag", [P, 1], F32, kind="ExternalInput").ap()
    glow = nc.dram_tensor("glow", [P, 1], F32, kind="ExternalInput").ap()
    mask4 = nc.dram_tensor("mask4", [P, 4 * 512], F32, kind="ExternalInput").ap()
    out = nc.dram_tensor("out", [HALF, H], F32, kind="ExternalOutput").ap()

    with tile.TileContext(nc) as tc, ExitStack() as ctx:
        sb = ctx.enter_context(tc.tile_pool(name="sb", bufs=4))
        big = ctx.enter_context(tc.tile_pool(name="big", bufs=1))
        ps2 = ctx.enter_context(tc.tile_pool(name="ps2", bufs=2, space="PSUM"))
        pqk = ctx.enter_context(tc.tile_pool(name="pqk", bufs=2, space="PSUM"))
        pvv = ctx.enter_context(tc.tile_pool(name="pvv", bufs=2, space="PSUM"))
        dram = ctx.enter_context(tc.tile_pool(name="dram", bufs=1, space="DRAM"))

        # ---- constants ----
        mask_sb = big.tile([P, 4 * 512], F32, tag="mask")
        nc.scalar.dma_start(mask_sb[:], mask4[:])
        flag_sb = big.tile([P, 1], F32, tag="flag")
        nc.scalar.dma_start(flag_sb[:], flag[:])
        glow_sb = big.tile([P, 1], F32, tag="glow")
        nc.scalar.dma_start(glow_sb[:], glow[:])
        wqk_sb = big.tile([P, NB_C, 2 * H], BF16, tag="wqk")
        nc.gpsimd.dma_start(wqk_sb[:, :, 0:H], wq.rearrange("(cb p) h -> p cb h", p=P))
        nc.gpsimd.dma_start(wqk_sb[:, :, H:2 * H], wk.rearrange("(cb p) h -> p cb h", p=P))
        wv_sb = big.tile([P, NB_C, H], BF16, tag="wv")
        nc.gpsimd.dma_start(wv_sb[:], wv.rearrange("(cb p) h -> p cb h", p=P))

        schedule = SCHEDULE if SCHEDULE is not None else [PHASES] * BODY_REPEAT
        for _rep in range(len(schedule)):
            cur = schedule[_rep]
            if 1 in cur or 2 in cur:
                xT = big.tile([P, NB_C, HALF], BF16, tag="xT")
                qk2q = big.tile([P, HALF], BF16, tag="qk2q")
                qk2k = big.tile([P, HALF], BF16, tag="qk2k")
                v_sb = big.tile([P, NT, H + 2], BF16, tag="v")

            if 2 in cur:
                nc.vector.memset(v_sb[:, :, H:H + 2], 1.0)

            if 1 in cur:
                xbf = dram.tile([HALF, C], BF16)
            for half in range(2):
                lo = half * 1024
                if 1 in cur:
                    nc.gpsimd.dma_start(xbf[lo:lo + 1024, :], x[lo:lo + 1024, :])
                    for cb in range(NB_C):
                        nc.sync.dma_start(
                            xT[:, cb, lo:lo + 1024],
                            xbf[lo:lo + 1024, cb * P:(cb + 1) * P],
                            transpose=True)
                if 2 in cur:
                    for tg in (2 * half, 2 * half + 1):
                        pq = pqk.tile([P, 512], F32, tag="pqk")
                        for cb in range(NB_C):
                            nc.tensor.matmul(pq[:], wqk_sb[:, cb, :],
                                             xT[:, cb, tg * 512:(tg + 1) * 512],
                                             start=(cb == 0), stop=(cb == NB_C - 1))
                        nc.vector.tensor_copy(qk2q[0:H, tg * 512:(tg + 1) * 512],
                                              pq[0:H, :])
                        nc.vector.tensor_copy(qk2k[H:P, tg * 512:(tg + 1) * 512],
                                              pq[H:P, :])
                    for sg in (2 * half, 2 * half + 1):
                        pv4 = pvv.tile([P, 4, H], F32, tag="pvv")
                        for j in range(4):
                            st = 4 * sg + j
                            for cb in range(NB_C):
                                nc.tensor.matmul(pv4[:, j, :],
                                                 xT[:, cb, st * P:(st + 1) * P],
                                                 wv_sb[:, cb, :],
                                                 start=(cb == 0), stop=(cb == NB_C - 1))
                        nc.vector.tensor_copy(
                            v_sb[:, 4 * sg:4 * sg + 4, 0:H], pv4[:])
                    # duplicate q/k into the other partition half for
                    # row-tiled QK pairs (SBUF->SBUF partition remap DMA)
                    nc.sync.dma_start(qk2q[H:P, lo:lo + 1024], qk2q[0:H, lo:lo + 1024])
                    nc.sync.dma_start(qk2k[0:H, lo:lo + 1024], qk2k[H:P, lo:lo + 1024])

                if 3 in cur:
                    nkv = KVQ_N if half == 0 else KV_K + KV_V
                    kvq = dram.tile([nkv], BF16)
                    nc.gpsimd.dma_start(
                        kvq[0:KV_K].rearrange("(p t) -> p t", p=H),
                        qk2k[H:P, lo:lo + 1024])
                    nc.gpsimd.dma_start(
                        kvq[KV_K:KV_K + KV_V].rearrange(
                            "(p st h) -> p st h", p=P, st=NT // 2),
                        v_sb[:, 8 * half:8 * half + 8, :])
                    if half == 0:
                        nc.gpsimd.dma_start(
                            kvq[KV_K + KV_V:KVQ_N].rearrange("(p t) -> p t", p=H),
                            qk2q[0:H, lo:lo + 1024])
                    gkv = dram.tile([2, nkv], BF16)
                    nc.gpsimd.collective_compute(
                        "AllGather", mybir.AluOpType.bypass,
                        replica_groups=PAIRS,
                        ins=[kvq.opt()], outs=[gkv.opt()])
                    KR2 = big.tile([P, HALF], BF16, tag="KR2")
                    nc.gpsimd.dma_start(
                        KR2[0:H, lo:lo + 1024],
                        gkv[0, 0:KV_K].rearrange("(p t) -> p t", p=H))
                    nc.gpsimd.dma_start(
                        KR2[H:P, lo:lo + 1024],
                        gkv[0, 0:KV_K].rearrange("(p t) -> p t", p=H))
                    VR = big.tile([P, NT, H + 2], BF16, tag="VR")
                    nc.gpsimd.dma_start(
                        VR[:, 8 * half:8 * half + 8, :],
                        gkv[0, KV_K:KV_K + KV_V].rearrange(
                            "(p st h) -> p st h", p=P, st=NT // 2))
                    if half == 0:
                        qr0 = big.tile([H, 1024], BF16, tag="qr0")
                        nc.gpsimd.dma_start(
                            qr0[:], gkv[1, KV_K + KV_V:KVQ_N].rearrange(
                                "(p t) -> p t", p=H))

            if 3 in cur:
                # QR = glow*qr0 + flag*qr1, duplicated to partitions 64:128
                QRb = big.tile([P, 1024], BF16, tag="QRb")
                qt1 = sb.tile([H, 1024], F32, tag="qt1")
                nc.vector.tensor_scalar_mul(qt1[:], qr0[:], glow_sb[0:H, 0:1])
                nc.vector.scalar_tensor_tensor(
                    QRb[0:H, :], qk2q[0:H, 1024:2048], flag_sb[0:H, 0:1], qt1[:],
                    op0=mybir.AluOpType.mult, op1=mybir.AluOpType.add)
                nc.sync.dma_start(QRb[H:P, :], QRb[0:H, :])

            if 4 in cur:
                # ---- tri QK row-tiled pairs + exp (N=1024) ----
                attT_tri = big.tile([P, TRI_TOTAL], BF16, tag="att_tri")
                for q in range(4):
                    w = TRI_W[4 * q]
                    for g in range(q, 4):
                        for jp in range(2):
                            stA = 4 * q + 2 * jp
                            pst = ps2.tile([P, 2, 512], F32, tag="ps2")
                            nc.tensor.matmul(
                                pst[:, 0, :],
                                qk2k[0:H, stA * P:(stA + 1) * P],
                                qk2q[0:H, g * 512:(g + 1) * 512],
                                start=True, stop=True)
                            nc.tensor.matmul(
                                pst[:, 1, :],
                                qk2k[H:P, (stA + 1) * P:(stA + 2) * P],
                                qk2q[H:P, g * 512:(g + 1) * 512],
                                start=True, stop=True)
                            if g == q:
                                nc.vector.tensor_add(
                                    pst[:], pst[:],
                                    mask_sb[:, (2 * jp) * 512:(2 * jp + 2) * 512]
                                    .rearrange("p (j c) -> p j c", j=2))
                            # strided dst view: tiles (stA, stA+1), width w
                            base = TRI_OFF[4 * q + 2 * jp] + (g - q) * 512
                            dst = attT_tri[:, base:base + 2 * w] \
                                .rearrange("p (j c) -> p j c", c=w)[:, :, 0:512]
                            nc.scalar.activation(
                                dst, pst[:],
                                mybir.ActivationFunctionType.Exp, scale=SCALE)

            if 5 in cur:
                # ---- rect QK row-tiled pairs + exp (N=1024) ----
                attT_rect = big.tile([P, NT, 1024], BF16, tag="att_rect")
                for sq in range(4):
                    for g in range(2):
                        for jp in range(2):
                            stA = 4 * sq + 2 * jp
                            psr = ps2.tile([P, 2, 512], F32, tag="ps2")
                            nc.tensor.matmul(
                                psr[:, 0, :],
                                KR2[0:H, stA * P:(stA + 1) * P],
                                QRb[0:H, g * 512:(g + 1) * 512],
                                start=True, stop=True)
                            nc.tensor.matmul(
                                psr[:, 1, :],
                                KR2[H:P, (stA + 1) * P:(stA + 2) * P],
                                QRb[H:P, g * 512:(g + 1) * 512],
                                start=True, stop=True)
                            nc.scalar.activation(
                                attT_rect[:, stA:stA + 2, g * 512:(g + 1) * 512],
                                psr[:],
                                mybir.ActivationFunctionType.Exp, scale=SCALE)

            if 6 in cur:
                # ---- tri AV (attT-stationary, rhs v with ones col) ----
                trind = big.tile([P, NT, H + 2], F32, tag="trind")
                for tp in range(8):
                    po = pvv.tile([P, 2, H + 2], F32, tag="pvv")
                    for j in range(2):
                        tt = 2 * tp + j
                        for st in range(tt + 1):
                            col = TRI_OFF[st] + tt * P - TRI_BASE[st]
                            nc.tensor.matmul(po[:, j, 0:H + 1],
                                             attT_tri[:, col:col + P],
                                             v_sb[:, st, 0:H + 1],
                                             start=(st == 0), stop=(st == tt))
                    nc.vector.tensor_copy(trind[:, 2 * tp:2 * tp + 2, 0:H + 1],
                                          po[:, :, 0:H + 1])

            if 7 in cur:
                # ---- rect AV + AG-3 ----
                rectnd = big.tile([P, NT // 2, H + 2], F32, tag="rectnd")
                for tp in range(4):
                    po2 = pvv.tile([P, 2, H + 2], F32, tag="pvv")
                    for j in range(2):
                        tt = 2 * tp + j
                        for st in range(NT):
                            nc.tensor.matmul(po2[:, j, 0:H + 1],
                                             attT_rect[:, st, tt * P:(tt + 1) * P],
                                             VR[:, st, 0:H + 1],
                                             start=(st == 0), stop=(st == NT - 1))
                    nc.vector.tensor_copy(rectnd[:, 2 * tp:2 * tp + 2, 0:H + 1],
                                          po2[:, :, 0:H + 1])
                ndd = dram.tile([ND_N], F32)
                nc.gpsimd.dma_start(
                    ndd.rearrange("(p st h) -> p st h", p=P, st=NT // 2),
                    rectnd[:])
                gnd = dram.tile([2, ND_N], F32)
                nc.gpsimd.collective_compute(
                    "AllGather", mybir.AluOpType.bypass,
                    replica_groups=PAIRS,
                    ins=[ndd.opt()], outs=[gnd.opt()])
                recvnd = big.tile([P, NT // 2, H + 2], F32, tag="recvnd")
                nc.gpsimd.dma_start(
                    recvnd[:], gnd[0].rearrange(
                        "(p st h) -> p st h", p=P, st=NT // 2))

            if 8 in cur:
                ndf = big.tile([P, NT, H + 1], F32, tag="ndf")
                nc.vector.scalar_tensor_tensor(
                    ndf[:, 0:8, :], recvnd[:, :, 0:H + 1], flag_sb[:, 0:1],
                    trind[:, 0:8, 0:H + 1],
                    op0=mybir.AluOpType.mult, op1=mybir.AluOpType.add)
                nc.vector.scalar_tensor_tensor(
                    ndf[:, 8:16, :], rectnd[:, :, 0:H + 1], flag_sb[:, 0:1],
                    trind[:, 8:16, 0:H + 1],
                    op0=mybir.AluOpType.mult, op1=mybir.AluOpType.add)
                rec = sb.tile([P, NT], F32, tag="rec")
                nc.vector.reciprocal(rec[:], ndf[:, :, H:H + 1])
                ot_all = big.tile([P, NT, H], F32, tag="ot_all")
                for tt in range(NT):
                    nc.vector.tensor_scalar_mul(
                        ot_all[:, tt, :], ndf[:, tt, 0:H], rec[:, tt:tt + 1])
                nc.sync.dma_start(
                    out.rearrange("(tt p) h -> p tt h", p=P), ot_all[:])

    nc.compile()
    return nc


def make_in_maps(x, Wq, Wk, Wv):
    x = np.asarray(x, dtype=np.float32)
    Wq = np.asarray(Wq, dtype=np.float32)
    Wk = np.asarray(Wk, dtype=np.float32)
    Wv = np.asarray(Wv, dtype=np.float32)
    # mask4[row, j*512 + col] = 0 if col >= j*128 + row else NEG
    rows = np.arange(P)[:, None]
    cols = np.arange(512)[None, :]
    m4 = np.concatenate(
        [np.where(cols >= j * P + rows, 0.0, NEG) for j in range(4)],
        axis=1).astype(np.float32)
    in_maps = []
    for c in range(N_CORES):
        b, h = c // 2, c % 2
        in_maps.append({
            "x": np.ascontiguousarray(x[b, h * HALF:(h + 1) * HALF, :]),
            "wq": Wq, "wk": Wk, "wv": Wv,
            "flag": np.full((P, 1), float(h), np.float32),
            "glow": np.full((P, 1), 1.0 - float(h), np.float32),
            "mask4": m4,
        })
    return in_maps


def kernel(x, Wq, Wk, Wv):
    if "nc" not in _CACHE:
        _CACHE["nc"] = build()
    nc = _CACHE["nc"]
    in_maps = make_in_maps(x, Wq, Wk, Wv)
    res = None
    for attempt in range(4):
        try:
            res = run_bass_kernel_spmd(nc, in_maps, list(range(N_CORES)))
            break
        except Exception:
            if attempt == 3:
                raise
            import time as _time
            _time.sleep(5)
    out = np.empty((B, T, H), np.float32)
    for c in range(N_CORES):
        b, h = c // 2, c % 2
        out[b, h * HALF:(h + 1) * HALF, :] = res.results[c]["out"]
    return out
